# revision 22
# baseline (speedup 1.0000x reference)
"""Trainium2 Bass kernel for nn_ConnectionTransformer (8 NeuronCores, SPMD).

Strategy
--------
- Phase A (embed + compress attention): batch-parallel, core c handles batch c.
- Phase B (6 bilinear message-passing steps): target-slot sharding — core c owns
  16 target slots j in [16c, 16c+16). Per-pair weights stream from HBM in fp16
  (halves the memory roofline vs fp32). Source slots are processed in groups
  g of 4 (i = 4g+il): einsum1 writes inter[(il,r), k, jloc, b] so einsum2 can
  contract over a full 128-partition (il,r) axis with the big W_target tile as
  the moving operand — 4x fewer PE cycles than a rank-32 contraction.
  Each step ends with relu/residual/LayerNorm on the 16 local slots and an
  AllGather that rebuilds the replicated transposed state h^T.
- Phase C (expand attention + vocab projection, fp16 W_out): batch-parallel.

All weights are pre-transposed/tiled on the host into matmul-ready layouts so
the device never transposes weight tensors.
"""
import os
import sys

sys.path.insert(0, "/opt/trn_rl_repo")

import numpy as np
from concourse import bass, bacc, tile, bass_utils, mybir
from concourse import masks

B, L, D, S, R, STEPS, V = 8, 512, 256, 128, 32, 6, 32000
NC = 8
JL = S // NC          # 16 local target slots per core
K = JL // 4           # 4 quads of target slots
G = S // 4            # 32 groups of 4 source slots
VT = 500              # vocab tile width
NVT = V // VT         # 64 vocab tiles
SCALE = 1.0 / np.sqrt(D)
LN_EPS = 1e-5

F32 = mybir.dt.float32
F16 = mybir.dt.float16

N_STEPS = int(os.environ.get("N_STEPS", str(STEPS)))
WBUFS = int(os.environ.get("WBUFS", "5"))
NVB = 16              # vocab blocks of 4*VT for W_out streaming


# ---------------------------------------------------------------------------
# Device program
# ---------------------------------------------------------------------------

def build():
    nc = bacc.Bacc("TRN2", target_bir_lowering=False, debug=False, num_devices=NC)

    io = {}

    def inp(name, shape, dtype=F32):
        io[name] = nc.dram_tensor(name, shape, dtype, kind="ExternalInput").ap()

    inp("xT_in", [D, L])
    inp("maskw", [128, 4])
    for w in ("wqT", "wkslT", "wvT", "wqoT", "wkfT", "wvfT"):
        inp(w, [D, D])
    inp("hT_in", [D, S])
    inp("hn_in", [S, D])
    inp("lngb", [B, STEPS, 2 * D])
    # wcomb[g, dp, 0:4096]  = wsrc[(dt il k jl r)]
    # wcomb[g, dp, 4096:]   = wtgt[(k jl d)]   (dp = (il r) for the wtgt half)
    inp("wcomb", [G, 128, 8192], F16)
    inp("woutT", [NVB, 2, 128, 4 * VT], F16)
    io["lg_out"] = nc.dram_tensor(
        "lg_out", [L, V], F16, kind="ExternalOutput"
    ).ap()
    io["dbg"] = nc.dram_tensor(
        "dbg", [128, 2048], F32, kind="ExternalOutput"
    ).ap()

    with tile.TileContext(nc) as tc:
        _body(nc, tc, io)
    nc.compile()
    return nc


def _body(nc, tc, io):
    with tc.tile_pool(name="const", bufs=1) as const, \
         tc.tile_pool(name="state", bufs=1) as state, \
         tc.tile_pool(name="ws", bufs=WBUFS) as wsp, \
         tc.tile_pool(name="inter", bufs=2) as intp, \
         tc.tile_pool(name="st_sb", bufs=1) as ssb, \
         tc.tile_pool(name="ag_dram", bufs=1, space="DRAM") as drp:

        ident = const.tile([128, 128], F32)
        masks.make_identity(nc, ident[:])
        ones = const.tile([128, 1], F32)
        nc.vector.memset(ones[:], 1.0)
        eps_sb = const.tile([128, 1], F32)
        nc.vector.memset(eps_sb[:], LN_EPS)

        pid = nc.sync.partition_id()

        # persistent state (replicated h^T lives in fp16 — einsum1 operand)
        hT = [state.tile([128, S, B], F16, name=f"hT{dt}") for dt in range(2)]
        h_upd = state.tile([B, JL, D], F32)      # this core's 16 slots
        qoT = [state.tile([128, L], F32, name=f"qoT{pt}") for pt in range(2)]
        lngb_sb = state.tile([B, STEPS, 2 * D], F32)
        nc.sync.dma_start(lngb_sb[:], io["lngb"])

        _phase_a(nc, tc, io, ident, ones, pid, hT, h_upd, qoT)
        with tc.tile_pool(name="ip_ps", bufs=2, space="PSUM") as ipp, \
             tc.tile_pool(name="acc_ps", bufs=1, space="PSUM") as accp, \
             tc.tile_pool(name="tp_ps", bufs=2, space="PSUM") as tpp:
            for t in range(N_STEPS):
                _step(nc, tc, t, io, hT, h_upd, lngb_sb, ident, eps_sb,
                      wsp, intp, ssb, ipp, accp, tpp, drp)
        _phase_c(nc, tc, io, ident, pid, hT, qoT)


def _phase_a(nc, tc, io, ident, ones, pid, hT, h_upd, qoT):
    with tc.tile_pool(name="pa_sb", bufs=1) as pa, \
         tc.tile_pool(name="pa_ps", bufs=3, space="PSUM") as pps, \
         tc.tile_pool(name="pa_tp", bufs=2, space="PSUM") as tps, \
         tc.tile_pool(name="pa_acc", bufs=1, space="PSUM") as aps, \
         tc.tile_pool(name="dram_a", bufs=1, space="DRAM") as dra:

        mask_sb = pa.tile([128, 4], F32)
        nc.sync.dma_start(mask_sb[:], io["maskw"])

        # X^T tiles [d128, t512] (host-gathered embeddings, transposed)
        xT = [pa.tile([128, L], F32, name=f"xT{ct}") for ct in range(2)]
        for ct in range(2):
            nc.sync.dma_start(xT[ct][:], io["xT_in"][128 * ct : 128 * (ct + 1), :])

        # weight tiles [d128, 256] (contraction on partitions)
        def load_w(name):
            ts = [pa.tile([128, D], F32, name=f"{name}_{ct}") for ct in range(2)]
            for ct in range(2):
                nc.sync.dma_start(ts[ct][:], io[name][128 * ct : 128 * (ct + 1), :])
            return ts

        wq_sb = load_w("wqT")
        wv_sb = load_w("wvT")
        wksl_sb = load_w("wkslT")
        wqo_sb = load_w("wqoT")
        hTt = [pa.tile([128, S], F32, name=f"hTt{ct}") for ct in range(2)]
        for ct in range(2):
            nc.sync.dma_start(hTt[ct][:], io["hT_in"][128 * ct : 128 * (ct + 1), :])
        hn_sb = pa.tile([S, D], F32)
        nc.sync.dma_start(hn_sb[:], io["hn_in"])

        # Q_in^T and Q_out^T : [d'128 x 2, t512]
        qT = [pa.tile([128, L], F32, name=f"qT{pt}") for pt in range(2)]
        for pt in range(2):
            for dst, wsb in ((qT, wq_sb), (qoT, wqo_sb)):
                ps = pps.tile([128, L], F32, tag="ps")
                for ct in range(2):
                    nc.tensor.matmul(
                        ps[:], wsb[ct][:, 128 * pt : 128 * (pt + 1)], xT[ct][:],
                        start=(ct == 0), stop=(ct == 1),
                    )
                nc.vector.tensor_copy(dst[pt][:], ps[:])

        # V_in natural [t128 x 4, d256]
        vn = pa.tile([128, 4, D], F32)
        for tt in range(4):
            ps = pps.tile([128, L], F32, tag="ps")
            for ct in range(2):
                nc.tensor.matmul(
                    ps[:, 0:D], xT[ct][:, 128 * tt : 128 * (tt + 1)], wv_sb[ct][:],
                    start=(ct == 0), stop=(ct == 1),
                )
            nc.vector.tensor_copy(vn[:, tt, :], ps[:, 0:D])

        # K_slots^T [d'128 x 2, s128]
        kslT = [pa.tile([128, S], F32, name=f"kslT{pt}") for pt in range(2)]
        for pt in range(2):
            ps = pps.tile([128, L], F32, tag="ps")
            for ct in range(2):
                nc.tensor.matmul(
                    ps[:, 0:S], wksl_sb[ct][:, 128 * pt : 128 * (pt + 1)], hTt[ct][:],
                    start=(ct == 0), stop=(ct == 1),
                )
            nc.vector.tensor_copy(kslT[pt][:], ps[:, 0:S])

        # attention scores + masked softmax
        a_sb = pa.tile([128, 4, S], F32)
        for tt in range(4):
            sc = pps.tile([128, L], F32, tag="ps")
            for pt in range(2):
                nc.tensor.matmul(
                    sc[:, 0:S], qT[pt][:, 128 * tt : 128 * (tt + 1)], kslT[pt][:],
                    start=(pt == 0), stop=(pt == 1),
                )
            rowmax = pa.tile([128, 1], F32, tag="rmax")
            nc.vector.tensor_reduce(
                rowmax[:], sc[:, 0:S], axis=mybir.AxisListType.X,
                op=mybir.AluOpType.max,
            )
            nb = pa.tile([128, 1], F32, tag="nb")
            nc.vector.tensor_scalar_mul(nb[:], rowmax[:], -SCALE)
            sumexp = pa.tile([128, 1], F32, tag="sexp")
            nc.scalar.activation(
                a_sb[:, tt, :], sc[:, 0:S], mybir.ActivationFunctionType.Exp,
                bias=nb[:], scale=SCALE, accum_out=sumexp[:],
            )
            rs = pa.tile([128, 1], F32, tag="rs")
            nc.vector.reciprocal(rs[:], sumexp[:])
            rm = pa.tile([128, 1], F32, tag="rmk")
            nc.vector.tensor_tensor(
                rm[:], rs[:], mask_sb[:, tt : tt + 1], op=mybir.AluOpType.mult
            )
            nc.vector.tensor_scalar_mul(a_sb[:, tt, :], a_sb[:, tt, :], rm[:])

        # column sums and IR = A^T @ V
        cs = aps.tile([128, 1], F32, tag="cs")
        for tt in range(4):
            nc.tensor.matmul(
                cs[:], a_sb[:, tt, :], ones[:, 0:1], start=(tt == 0), stop=(tt == 3)
            )
        ir = aps.tile([128, D], F32, tag="ir")
        for tt in range(4):
            nc.tensor.matmul(
                ir[:], a_sb[:, tt, :], vn[:, tt, :], start=(tt == 0), stop=(tt == 3)
            )
        cssb = pa.tile([128, 1], F32)
        nc.vector.tensor_scalar_add(cssb[:], cs[:], 1e-8)
        rcs = pa.tile([128, 1], F32)
        nc.vector.reciprocal(rcs[:], cssb[:])
        h0 = pa.tile([S, D], F32)
        nc.vector.scalar_tensor_tensor(
            h0[:], ir[:], rcs[:], hn_sb[:],
            op0=mybir.AluOpType.mult, op1=mybir.AluOpType.add,
        )

        # h0 -> transposed bounce + natural bounce, init AllGather
        agin0 = dra.tile([4 * 16384], F32)
        for dt in range(2):
            p3 = tps.tile([128, 128], F32, tag="tp")
            nc.tensor.transpose(p3[:], h0[:, 128 * dt : 128 * (dt + 1)], ident[:])
            h0T = pa.tile([128, 128], F32, tag="h0T")
            nc.vector.tensor_copy(h0T[:], p3[:])
            nc.sync.dma_start(
                agin0[dt * 16384 : (dt + 1) * 16384].rearrange(
                    "(p f) -> p f", p=128
                ),
                h0T[:],
            )
        nc.sync.dma_start(
            agin0[32768:65536].rearrange("(p f) -> p f", p=128), h0[:]
        )
        agout0 = dra.tile([NC, 4 * 16384], F32, addr_space="Shared")
        nc.gpsimd.collective_compute(
            "AllGather", mybir.AluOpType.bypass,
            ins=[agin0[:].opt()], outs=[agout0[:].opt()],
            replica_groups=[list(range(NC))],
        )
        # readback: hT[dt][dp, s, b] ; h_upd[b, jl, d] (own slot range via pid)
        ag0r = agout0[:].rearrange(
            "b (seg dp s) -> seg dp s b", seg=4, dp=128, s=128
        )
        for dt in range(2):
            hT32 = pa.tile([128, S, B], F32, tag="hT32")
            nc.sync.dma_start(hT32[:], ag0r[dt])
            nc.vector.tensor_copy(hT[dt][:], hT32[:])
        nc.sync.dma_start(
            h_upd[:].rearrange("b jl d -> b (jl d)"),
            agout0[:][:, bass.ds(pid * (JL * D) + 32768, JL * D)],
        )


def _step(nc, tc, t, io, hT, h_upd, lngb_sb, ident, eps_sb,
          wsp, intp, ssb, ipp, accp, tpp, drp):
    """One message-passing step (fp16 weights, grouped source slots)."""
    # per-(k) influence accumulators: acc[k][32*jl + b, d]
    acc = [accp.tile([128, D], F32, tag=f"acc{k}", name=f"acc{k}") for k in range(4)]

    for g in range(G):
        # one 2 MB fp16 stream per source-slot quad, on the ACT HWDGE ring
        # (keeps weight prefetch flowing past collective waits on sync)
        wcb = wsp.tile([128, 8192], F16, tag="w")
        nc.scalar.dma_start(wcb[:], io["wcomb"][g])

        # einsum1: ip[(il r), k, jloc, b] = sum_d h[b, 4g+il, d] Ws[d, r]
        ip = ipp.tile([128, 4, 4, B], F32, tag="ip")
        for il in range(4):
            i = 4 * g + il
            for k in range(4):
                for jl in range(4):
                    for dt in range(2):
                        off = (((dt * 4 + il) * 4 + k) * 4 + jl) * R
                        nc.tensor.matmul(
                            ip[32 * il : 32 * (il + 1), k, jl, :],
                            wcb[:, off : off + R],
                            hT[dt][:, i, :],
                            start=(dt == 0), stop=(dt == 1),
                            tile_position=(0, 32 * il),
                        )
        inter = intp.tile([128, 4, 4, B], F16, tag="inter")
        nc.vector.tensor_copy(inter[:], ip[:])

        # einsum2: acc[k][32 jl + b, d] += inter[:, k, jl, :]^T @ Wt[:, k, jl, :]
        for k in range(4):
            for jl in range(4):
                off = 4096 + (k * 4 + jl) * D
                nc.tensor.matmul(
                    acc[k][32 * jl : 32 * jl + B, :],
                    inter[:, k, jl, :],
                    wcb[:, off : off + D],
                    start=(g == 0), stop=(g == G - 1),
                    tile_position=(0, 32 * jl),
                )

    # ---- relu / residual / LayerNorm per quad k ----
    hTloc = [ssb.tile([128, JL * B], F16, name=f"hTl{dt}") for dt in range(2)]
    for k in range(4):
        hrelu = ssb.tile([B, 4, D], F32, tag="hrelu")
        for jl in range(4):
            nc.scalar.activation(
                hrelu[:, jl, :], acc[k][32 * jl : 32 * jl + B, :],
                mybir.ActivationFunctionType.Relu,
            )
        hsum = ssb.tile([B, 4, D], F32, tag="hsum")
        nc.vector.tensor_tensor(
            hsum[:], hrelu[:], h_upd[:, 4 * k : 4 * (k + 1), :],
            op=mybir.AluOpType.add,
        )
        mean = ssb.tile([B, 4], F32, tag="mean")
        nc.vector.tensor_reduce(
            mean[:], hsum[:], axis=mybir.AxisListType.X, op=mybir.AluOpType.add
        )
        nc.vector.tensor_scalar_mul(mean[:], mean[:], 1.0 / D)
        cen = ssb.tile([B, 4, D], F32, tag="cen")
        nc.vector.tensor_tensor(
            cen[:], hsum[:], mean[:].to_broadcast((B, 4, D)),
            op=mybir.AluOpType.subtract,
        )
        sq = ssb.tile([B, 4, D], F32, tag="sq")
        nc.vector.tensor_tensor(
            sq[:], cen[:], cen[:], op=mybir.AluOpType.mult
        )
        var = ssb.tile([B, 4], F32, tag="var")
        nc.vector.tensor_reduce(
            var[:], sq[:], axis=mybir.AxisListType.X, op=mybir.AluOpType.add
        )
        std = ssb.tile([B, 4], F32, tag="std")
        nc.scalar.activation(
            std[:], var[:], mybir.ActivationFunctionType.Sqrt,
            bias=eps_sb[0:B, :], scale=1.0 / D,
        )
        rstd = ssb.tile([B, 4], F32, tag="rstd")
        nc.vector.reciprocal(rstd[:], std[:])
        hnorm = ssb.tile([B, 4, D], F32, tag="hnorm")
        nc.vector.tensor_tensor(
            hnorm[:], cen[:], rstd[:].to_broadcast((B, 4, D)),
            op=mybir.AluOpType.mult,
        )
        g_b = lngb_sb[:, t : t + 1, 0:D].to_broadcast((B, 4, D))
        b_b = lngb_sb[:, t : t + 1, D : 2 * D].to_broadcast((B, 4, D))
        nc.vector.tensor_tensor(
            hnorm[:], hnorm[:], g_b, op=mybir.AluOpType.mult
        )
        nc.vector.tensor_tensor(
            h_upd[:, 4 * k : 4 * (k + 1), :], hnorm[:], b_b,
            op=mybir.AluOpType.add,
        )
        # transpose the 4 updated slots into hTloc (fp32 transpose, f16 store)
        for jloc in range(4):
            jl = 4 * k + jloc
            for dt in range(2):
                p3 = tpp.tile([128, B], F32, tag="p3")
                nc.tensor.transpose(
                    p3[:],
                    h_upd[:, jl, 128 * dt : 128 * (dt + 1)],
                    ident[0:B, 0:B],
                )
                nc.vector.tensor_copy(
                    hTloc[dt][:, jl * B : (jl + 1) * B], p3[:]
                )

    # ---- AllGather the transposed updated slots (fp16); rebuild hT ----
    agin = drp.tile([2 * 128 * JL * B], F16, tag=f"agin{t}")
    for dt in range(2):
        nc.sync.dma_start(
            agin[dt * 16384 : (dt + 1) * 16384].rearrange(
                "(p f) -> p f", p=128
            ),
            hTloc[dt][:],
        )
    agout = drp.tile([NC, 2 * 128 * JL * B], F16, addr_space="Shared",
                     tag=f"agout{t}")
    nc.gpsimd.collective_compute(
        "AllGather", mybir.AluOpType.bypass,
        ins=[agin[:].opt()], outs=[agout[:].opt()],
        replica_groups=[list(range(NC))],
    )
    agr = agout[:].rearrange(
        "rk (dt dp jl b) -> dt dp rk jl b", dt=2, dp=128, jl=JL, b=B
    )
    for dt in range(2):
        nc.sync.dma_start(
            hT[dt][:].rearrange("dp (rk jl) b -> dp rk jl b", rk=NC), agr[dt]
        )


def _phase_c(nc, tc, io, ident, pid, hT, qoT):
    with tc.tile_pool(name="pc_sb", bufs=1) as pc, \
         tc.tile_pool(name="pc_ps", bufs=3, space="PSUM") as cps, \
         tc.tile_pool(name="pc_lg", bufs=4, space="PSUM") as lgps, \
         tc.tile_pool(name="pc_wo", bufs=4) as wop:

        wkf_sb = [pc.tile([128, D], F32, name=f"wkf{ct}") for ct in range(2)]
        wvf_sb = [pc.tile([128, D], F32, name=f"wvf{ct}") for ct in range(2)]
        for ct in range(2):
            nc.sync.dma_start(
                wkf_sb[ct][:], io["wkfT"][128 * ct : 128 * (ct + 1), :]
            )
            nc.sync.dma_start(
                wvf_sb[ct][:], io["wvfT"][128 * ct : 128 * (ct + 1), :]
            )

        # own-batch h^T slice (dynamic b=pid) -> static tiles
        pid_v = nc.vector.partition_id()
        hb = [pc.tile([128, S], F32, name=f"hb{dt}") for dt in range(2)]
        for dt in range(2):
            nc.vector.tensor_copy(
                hb[dt][:].rearrange("p (s o) -> p s o", o=1),
                hT[dt][:, :, bass.ds(pid_v, 1)],
            )

        # K_f^T [d'128 x2, s128] ; V_f natural [s, d']
        kfT = [pc.tile([128, S], F32, name=f"kfT{pt}") for pt in range(2)]
        for pt in range(2):
            ps = cps.tile([128, L], F32, tag="c")
            for ct in range(2):
                nc.tensor.matmul(
                    ps[:, 0:S], wkf_sb[ct][:, 128 * pt : 128 * (pt + 1)], hb[ct][:],
                    start=(ct == 0), stop=(ct == 1),
                )
            nc.vector.tensor_copy(kfT[pt][:], ps[:, 0:S])
        vf = pc.tile([S, D], F32)
        psv = cps.tile([128, L], F32, tag="c")
        for ct in range(2):
            nc.tensor.matmul(
                psv[0:S, 0:D], hb[ct][:], wvf_sb[ct][:],
                start=(ct == 0), stop=(ct == 1),
            )
        nc.vector.tensor_copy(vf[:], psv[0:S, 0:D])

        # expand attention -> A2^T [s, t512]
        a2T = pc.tile([S, L], F32)
        for tt in range(4):
            sc = cps.tile([128, L], F32, tag="c")
            for pt in range(2):
                nc.tensor.matmul(
                    sc[:, 0:S], qoT[pt][:, 128 * tt : 128 * (tt + 1)], kfT[pt][:],
                    start=(pt == 0), stop=(pt == 1),
                )
            rowmax = pc.tile([128, 1], F32, tag="rmax2")
            nc.vector.tensor_reduce(
                rowmax[:], sc[:, 0:S], axis=mybir.AxisListType.X,
                op=mybir.AluOpType.max,
            )
            nb = pc.tile([128, 1], F32, tag="nb2")
            nc.vector.tensor_scalar_mul(nb[:], rowmax[:], -SCALE)
            a2 = pc.tile([128, S], F32, tag="a2")
            sumexp = pc.tile([128, 1], F32, tag="sexp2")
            nc.scalar.activation(
                a2[:], sc[:, 0:S], mybir.ActivationFunctionType.Exp,
                bias=nb[:], scale=SCALE, accum_out=sumexp[:],
            )
            rs = pc.tile([128, 1], F32, tag="rs2")
            nc.vector.reciprocal(rs[:], sumexp[:])
            nc.vector.tensor_scalar_mul(a2[:], a2[:], rs[:])
            ptr = cps.tile([128, L], F32, tag="c")
            nc.tensor.transpose(ptr[:, 0:S], a2[:], ident[:])
            nc.vector.tensor_copy(a2T[:, 128 * tt : 128 * (tt + 1)], ptr[:, 0:S])

        # Y^T [d128 x2, t512] -> fp16 for the vocab matmuls
        yTh = [pc.tile([128, L], F16, name=f"yTh{dt}") for dt in range(2)]
        for dt in range(2):
            ps = cps.tile([128, L], F32, tag="c")
            nc.tensor.matmul(
                ps[:], vf[:, 128 * dt : 128 * (dt + 1)], a2T[:],
                start=True, stop=True,
            )
            nc.vector.tensor_copy(yTh[dt][:], ps[:])

        # logits tiles: 1 MB W_out blocks (4 vocab tiles), fp16 batched stores
        lg_outr = io["lg_out"].rearrange("(tt p) v -> p tt v", tt=4)
        for bv in range(NVB):
            wo_sb = wop.tile([128, 2, 4, VT], F16, tag="wo")
            nc.scalar.dma_start(
                wo_sb[:],
                io["woutT"][bv].rearrange("dt dp (q v) -> dp dt q v", q=4),
            )
            for q in range(4):
                vt = 4 * bv + q
                lg_sb = wop.tile([128, 4, VT], F16, tag="lg_sb", name="lg_sb")
                for tt in range(4):
                    lg = lgps.tile([128, VT], F32, tag="lg")
                    for dt in range(2):
                        nc.tensor.matmul(
                            lg[:],
                            yTh[dt][:, 128 * tt : 128 * (tt + 1)],
                            wo_sb[:, dt, q, :],
                            start=(dt == 0), stop=(dt == 1),
                        )
                    nc.any.tensor_copy(lg_sb[:, tt, :], lg[:])
                nc.sync.dma_start(
                    lg_outr[:, :, VT * vt : VT * (vt + 1)], lg_sb[:]
                )


# ---------------------------------------------------------------------------
# Host side
# ---------------------------------------------------------------------------

_NC_CACHE = {}


def _get_nc():
    key = N_STEPS
    if key not in _NC_CACHE:
        _NC_CACHE[key] = build()
    return _NC_CACHE[key]


def _prep_in_maps(inputs):
    f32 = lambda a: np.ascontiguousarray(np.asarray(a), dtype=np.float32)
    input_ids = np.asarray(inputs["input_ids"])
    attention_mask = np.asarray(inputs["attention_mask"])
    H = f32(inputs["H"])
    W_source = f32(inputs["W_source"])
    W_target = f32(inputs["W_target"])

    lngb = np.zeros((B, STEPS, 2 * D), dtype=np.float32)
    lngb[:, :, 0:D] = np.asarray(inputs["ln_scale"])[None]
    lngb[:, :, D:] = np.asarray(inputs["ln_bias"])[None]

    rep = {
        "wqT": f32(np.asarray(inputs["Wq_in"]).T),
        "wkslT": f32(np.asarray(inputs["Wk_slots"]).T),
        "wvT": f32(np.asarray(inputs["Wv_in"]).T),
        "wqoT": f32(np.asarray(inputs["Wq_out"]).T),
        "wkfT": f32(np.asarray(inputs["Wk_fin"]).T),
        "wvfT": f32(np.asarray(inputs["Wv_fin"]).T),
        "hT_in": f32(H.T),
        "hn_in": H,
        "lngb": lngb,
        # woutT[bv, dtile, dp, (q vl)] = Wout[500(4bv+q)+vl, 128dt+dp]
        "woutT": np.ascontiguousarray(
            f32(inputs["W_out_proj"])
            .reshape(NVB, 4, VT, 2, 128)
            .transpose(0, 3, 4, 1, 2)
            .reshape(NVB, 2, 128, 4 * VT)
        ).astype(np.float16),
    }

    in_maps = []
    for c in range(NC):
        m = dict(rep)
        X = (np.asarray(inputs["token_emb"], dtype=np.float32)[input_ids[c]]
             + np.asarray(inputs["pos_emb"], dtype=np.float32))
        m["xT_in"] = np.ascontiguousarray(X.T)
        m["maskw"] = np.ascontiguousarray(
            attention_mask[c].astype(np.float32).reshape(4, 128).T
        )
        # wsrc[g, dp, (dt il k jloc r)] = W_source[4g+il, 16c+4k+jloc, 128dt+dp, r]
        ws = W_source[:, JL * c : JL * (c + 1)]      # [S, 16, D, R]
        ws = ws.reshape(G, 4, 4, 4, 2, 128, R).transpose(0, 5, 4, 1, 2, 3, 6)
        ws = np.ascontiguousarray(ws).astype(np.float16).reshape(G, 128, 4096)
        # wtgt[g, (il r), (k jloc d)] = W_target[4g+il, 16c+4k+jloc, r, d]
        # The reference masks out the i == j (diagonal) pair; zeroing
        # W_target[j, j] is exactly equivalent since the term is linear in it.
        wt = W_target[:, JL * c : JL * (c + 1)].copy()   # [S, 16, R, D]
        for jl in range(JL):
            wt[JL * c + jl, jl] = 0.0
        wt = wt.reshape(G, 4, 4, 4, R, D).transpose(0, 1, 4, 2, 3, 5)
        wt = np.ascontiguousarray(wt).astype(np.float16).reshape(G, 128, 4096)
        m["wcomb"] = np.ascontiguousarray(
            np.concatenate([ws, wt], axis=2)
        )
        in_maps.append(m)
    return in_maps


def run(inputs, trace=False):
    nc = _get_nc()
    in_maps = _prep_in_maps(inputs)
    res = bass_utils.run_bass_kernel_spmd(
        nc, in_maps, core_ids=list(range(NC)), trace=trace
    )
    out = np.stack(
        [res.results[c]["lg_out"].astype(np.float32) for c in range(NC)], axis=0
    )
    return out, res


def kernel(**inputs):
    out, _ = run(inputs, trace=False)
    return out


# revision 33
# speedup vs baseline: 1.2375x; 1.2375x over previous
"""Trainium2 Bass kernel for nn_ConnectionTransformer (8 NeuronCores, SPMD).

Strategy
--------
- Phase A (embed + compress attention): batch-parallel, core c handles batch c.
- Phase B (6 bilinear message-passing steps): target-slot sharding — core c owns
  16 target slots j in [16c, 16c+16). Per-pair weights stream from HBM in fp16
  (halves the memory roofline vs fp32). Source slots are processed in groups
  g of 4 (i = 4g+il): einsum1 writes inter[(il,r), k, jloc, b] so einsum2 can
  contract over a full 128-partition (il,r) axis with the big W_target tile as
  the moving operand — 4x fewer PE cycles than a rank-32 contraction.
  Each step ends with relu/residual/LayerNorm on the 16 local slots and an
  AllGather that rebuilds the replicated transposed state h^T.
- Phase C (expand attention + vocab projection, fp16 W_out): batch-parallel.

All weights are pre-transposed/tiled on the host into matmul-ready layouts so
the device never transposes weight tensors.
"""
import os
import sys

sys.path.insert(0, "/opt/trn_rl_repo")

import numpy as np
from concourse import bass, bacc, tile, bass_utils, mybir
from concourse import masks

B, L, D, S, R, STEPS, V = 8, 512, 256, 128, 32, 6, 32000
NC = 8
JL = S // NC          # 16 local target slots per core
K = JL // 4           # 4 quads of target slots
G = S // 4            # 32 groups of 4 source slots
VT = 500              # vocab tile width
NVT = V // VT         # 64 vocab tiles
SCALE = 1.0 / np.sqrt(D)
LN_EPS = 1e-5

F32 = mybir.dt.float32
F16 = mybir.dt.float16

N_STEPS = int(os.environ.get("N_STEPS", str(STEPS)))
WBUFS = int(os.environ.get("WBUFS", "6"))
NVB = 16              # vocab blocks of 4*VT for W_out streaming


# ---------------------------------------------------------------------------
# Device program
# ---------------------------------------------------------------------------

def build():
    nc = bacc.Bacc("TRN2", target_bir_lowering=False, debug=False, num_devices=NC)

    io = {}

    def inp(name, shape, dtype=F32):
        io[name] = nc.dram_tensor(name, shape, dtype, kind="ExternalInput").ap()

    inp("xT_in", [D, L])
    inp("maskw", [128, 4])
    for w in ("wqT", "wkslT", "wvT", "wqoT", "wkfT", "wvfT"):
        inp(w, [D, D])
    inp("hT_in", [D, S])
    inp("hn_in", [S, D])
    inp("lngb", [128, STEPS, 2 * D])
    # wcomb[g, dp, 0:4096]  = wsrc[(dt il k jl r)]
    # wcomb[g, dp, 4096:]   = wtgt[(k jl d)]   (dp = (il r) for the wtgt half)
    inp("wcomb", [G, 128, 8192], F16)
    inp("woutT", [NVB, 2, 128, 4 * VT], F16)
    io["lg_out"] = nc.dram_tensor(
        "lg_out", [L, V], F16, kind="ExternalOutput"
    ).ap()
    io["dbg"] = nc.dram_tensor(
        "dbg", [128, 2048], F32, kind="ExternalOutput"
    ).ap()

    with tile.TileContext(nc) as tc:
        _body(nc, tc, io)
    nc.compile()
    return nc


def _body(nc, tc, io):
    with tc.tile_pool(name="const", bufs=1) as const, \
         tc.tile_pool(name="state", bufs=1) as state, \
         tc.tile_pool(name="ws", bufs=WBUFS) as wsp, \
         tc.tile_pool(name="inter", bufs=2) as intp, \
         tc.tile_pool(name="st_sb", bufs=1) as ssb, \
         tc.tile_pool(name="ag_dram", bufs=1, space="DRAM") as drp:

        ident = const.tile([128, 128], F32)
        masks.make_identity(nc, ident[:])
        ones = const.tile([128, 1], F32)
        nc.vector.memset(ones[:], 1.0)
        eps_sb = const.tile([128, 1], F32)
        nc.vector.memset(eps_sb[:], LN_EPS)

        pid = nc.sync.partition_id()

        # persistent state (replicated h^T lives in fp16 — einsum1 operand)
        hT = [state.tile([128, S, B], F16, name=f"hT{dt}") for dt in range(2)]
        # this core's 16 slots, in accumulator layout: h_upd[k][32 jl + b, d]
        h_upd = [state.tile([128, D], F32, name=f"hup{k}") for k in range(4)]
        qoT = [state.tile([128, L], F32, name=f"qoT{pt}") for pt in range(2)]
        lngb_sb = state.tile([128, STEPS, 2 * D], F32)
        nc.sync.dma_start(lngb_sb[:], io["lngb"])

        # software-pipelined weight stream: issue runs WBUFS-1 groups ahead
        # of consumption so the ACT HWDGE ring streams through phase A and
        # the per-step collective tails.
        wq = []

        def issue_w(gg):
            if gg >= N_STEPS * G:
                return
            wcb = wsp.tile([128, 8192], F16, tag="w")
            nc.scalar.dma_start(wcb[:], io["wcomb"][gg % G])
            wq.append(wcb)

        for gg in range(WBUFS - 1):
            issue_w(gg)

        _phase_a(nc, tc, io, ident, ones, pid, hT, h_upd, qoT)
        with tc.tile_pool(name="ip_ps", bufs=2, space="PSUM") as ipp, \
             tc.tile_pool(name="acc_ps", bufs=1, space="PSUM") as accp, \
             tc.tile_pool(name="tp_ps", bufs=2, space="PSUM") as tpp:
            for t in range(N_STEPS):
                _step(nc, tc, t, io, hT, h_upd, lngb_sb, ident, eps_sb,
                      wq, issue_w, intp, ssb, ipp, accp, tpp, drp)
        _phase_c(nc, tc, io, ident, pid, hT, qoT)


def _phase_a(nc, tc, io, ident, ones, pid, hT, h_upd, qoT):
    with tc.tile_pool(name="pa_sb", bufs=1) as pa, \
         tc.tile_pool(name="pa_ps", bufs=3, space="PSUM") as pps, \
         tc.tile_pool(name="pa_tp", bufs=2, space="PSUM") as tps, \
         tc.tile_pool(name="pa_acc", bufs=1, space="PSUM") as aps, \
         tc.tile_pool(name="dram_a", bufs=1, space="DRAM") as dra:

        mask_sb = pa.tile([128, 4], F32)
        nc.sync.dma_start(mask_sb[:], io["maskw"])

        # X^T tiles [d128, t512] (host-gathered embeddings, transposed)
        xT = [pa.tile([128, L], F32, name=f"xT{ct}") for ct in range(2)]
        for ct in range(2):
            nc.sync.dma_start(xT[ct][:], io["xT_in"][128 * ct : 128 * (ct + 1), :])

        # weight tiles [d128, 256] (contraction on partitions)
        def load_w(name):
            ts = [pa.tile([128, D], F32, name=f"{name}_{ct}") for ct in range(2)]
            for ct in range(2):
                nc.sync.dma_start(ts[ct][:], io[name][128 * ct : 128 * (ct + 1), :])
            return ts

        wq_sb = load_w("wqT")
        wv_sb = load_w("wvT")
        wksl_sb = load_w("wkslT")
        wqo_sb = load_w("wqoT")
        hTt = [pa.tile([128, S], F32, name=f"hTt{ct}") for ct in range(2)]
        for ct in range(2):
            nc.sync.dma_start(hTt[ct][:], io["hT_in"][128 * ct : 128 * (ct + 1), :])
        hn_sb = pa.tile([S, D], F32)
        nc.sync.dma_start(hn_sb[:], io["hn_in"])

        # Q_in^T and Q_out^T : [d'128 x 2, t512]
        qT = [pa.tile([128, L], F32, name=f"qT{pt}") for pt in range(2)]
        for pt in range(2):
            for dst, wsb in ((qT, wq_sb), (qoT, wqo_sb)):
                ps = pps.tile([128, L], F32, tag="ps")
                for ct in range(2):
                    nc.tensor.matmul(
                        ps[:], wsb[ct][:, 128 * pt : 128 * (pt + 1)], xT[ct][:],
                        start=(ct == 0), stop=(ct == 1),
                    )
                nc.vector.tensor_copy(dst[pt][:], ps[:])

        # V_in natural [t128 x 4, d256]
        vn = pa.tile([128, 4, D], F32)
        for tt in range(4):
            ps = pps.tile([128, L], F32, tag="ps")
            for ct in range(2):
                nc.tensor.matmul(
                    ps[:, 0:D], xT[ct][:, 128 * tt : 128 * (tt + 1)], wv_sb[ct][:],
                    start=(ct == 0), stop=(ct == 1),
                )
            nc.vector.tensor_copy(vn[:, tt, :], ps[:, 0:D])

        # K_slots^T [d'128 x 2, s128]
        kslT = [pa.tile([128, S], F32, name=f"kslT{pt}") for pt in range(2)]
        for pt in range(2):
            ps = pps.tile([128, L], F32, tag="ps")
            for ct in range(2):
                nc.tensor.matmul(
                    ps[:, 0:S], wksl_sb[ct][:, 128 * pt : 128 * (pt + 1)], hTt[ct][:],
                    start=(ct == 0), stop=(ct == 1),
                )
            nc.vector.tensor_copy(kslT[pt][:], ps[:, 0:S])

        # attention scores + masked softmax
        a_sb = pa.tile([128, 4, S], F32)
        for tt in range(4):
            sc = pps.tile([128, L], F32, tag="ps")
            for pt in range(2):
                nc.tensor.matmul(
                    sc[:, 0:S], qT[pt][:, 128 * tt : 128 * (tt + 1)], kslT[pt][:],
                    start=(pt == 0), stop=(pt == 1),
                )
            rowmax = pa.tile([128, 1], F32, tag="rmax")
            nc.vector.tensor_reduce(
                rowmax[:], sc[:, 0:S], axis=mybir.AxisListType.X,
                op=mybir.AluOpType.max,
            )
            nb = pa.tile([128, 1], F32, tag="nb")
            nc.vector.tensor_scalar_mul(nb[:], rowmax[:], -SCALE)
            sumexp = pa.tile([128, 1], F32, tag="sexp")
            nc.scalar.activation(
                a_sb[:, tt, :], sc[:, 0:S], mybir.ActivationFunctionType.Exp,
                bias=nb[:], scale=SCALE, accum_out=sumexp[:],
            )
            rs = pa.tile([128, 1], F32, tag="rs")
            nc.vector.reciprocal(rs[:], sumexp[:])
            rm = pa.tile([128, 1], F32, tag="rmk")
            nc.vector.tensor_tensor(
                rm[:], rs[:], mask_sb[:, tt : tt + 1], op=mybir.AluOpType.mult
            )
            nc.vector.tensor_scalar_mul(a_sb[:, tt, :], a_sb[:, tt, :], rm[:])

        # column sums and IR = A^T @ V
        cs = aps.tile([128, 1], F32, tag="cs")
        for tt in range(4):
            nc.tensor.matmul(
                cs[:], a_sb[:, tt, :], ones[:, 0:1], start=(tt == 0), stop=(tt == 3)
            )
        ir = aps.tile([128, D], F32, tag="ir")
        for tt in range(4):
            nc.tensor.matmul(
                ir[:], a_sb[:, tt, :], vn[:, tt, :], start=(tt == 0), stop=(tt == 3)
            )
        cssb = pa.tile([128, 1], F32)
        nc.vector.tensor_scalar_add(cssb[:], cs[:], 1e-8)
        rcs = pa.tile([128, 1], F32)
        nc.vector.reciprocal(rcs[:], cssb[:])
        h0 = pa.tile([S, D], F32)
        nc.vector.scalar_tensor_tensor(
            h0[:], ir[:], rcs[:], hn_sb[:],
            op0=mybir.AluOpType.mult, op1=mybir.AluOpType.add,
        )

        # h0 -> transposed bounce + natural bounce, init AllGather
        agin0 = dra.tile([4 * 16384], F32)
        for dt in range(2):
            p3 = tps.tile([128, 128], F32, tag="tp")
            nc.tensor.transpose(p3[:], h0[:, 128 * dt : 128 * (dt + 1)], ident[:])
            h0T = pa.tile([128, 128], F32, tag="h0T")
            nc.vector.tensor_copy(h0T[:], p3[:])
            nc.sync.dma_start(
                agin0[dt * 16384 : (dt + 1) * 16384].rearrange(
                    "(p f) -> p f", p=128
                ),
                h0T[:],
            )
        nc.sync.dma_start(
            agin0[32768:65536].rearrange("(p f) -> p f", p=128), h0[:]
        )
        agout0 = dra.tile([NC, 4 * 16384], F32, addr_space="Shared")
        nc.gpsimd.collective_compute(
            "AllGather", mybir.AluOpType.bypass,
            ins=[agin0[:].opt()], outs=[agout0[:].opt()],
            replica_groups=[list(range(NC))],
        )
        # readback. staging in [dp, b, s] keeps the DMA runs 512 B long
        # (b innermost would be 4 B element gathers); the DVE rearranges.
        ag0r = agout0[:].rearrange(
            "b (seg dp s) -> seg dp b s", seg=4, dp=128, s=128
        )
        for dt in range(2):
            hT32 = pa.tile([128, B, S], F32, tag="hT32")
            nc.sync.dma_start(hT32[:], ag0r[dt])
            nc.vector.tensor_copy(
                hT[dt][:], hT32[:].rearrange("p b s -> p s b")
            )
        # h_upd[k][32 jl + b, d] <- h0[16c + 4k + jl, d] of batch b's core
        for k in range(4):
            for jl in range(4):
                nc.sync.dma_start(
                    h_upd[k][32 * jl : 32 * jl + B, :],
                    agout0[:][
                        :,
                        bass.ds(
                            pid * (JL * D) + 32768 + (4 * k + jl) * D, D
                        ),
                    ],
                )


def _step(nc, tc, t, io, hT, h_upd, lngb_sb, ident, eps_sb,
          wq, issue_w, intp, ssb, ipp, accp, tpp, drp):
    """One message-passing step (fp16 weights, grouped source slots)."""
    # per-(k) influence accumulators: acc[k][32*jl + b, d]
    acc = [accp.tile([128, D], F32, tag=f"acc{k}", name=f"acc{k}") for k in range(4)]

    for g in range(G):
        wcb = wq.pop(0)
        issue_w(t * G + g + WBUFS - 1)

        # einsum1: ip[(il r), k, jloc, b] = sum_d h[b, 4g+il, d] Ws[d, r]
        ip = ipp.tile([128, 4, 4, B], F32, tag="ip")
        for il in range(4):
            i = 4 * g + il
            for k in range(4):
                for jl in range(4):
                    for dt in range(2):
                        off = (((dt * 4 + il) * 4 + k) * 4 + jl) * R
                        nc.tensor.matmul(
                            ip[32 * il : 32 * (il + 1), k, jl, :],
                            wcb[:, off : off + R],
                            hT[dt][:, i, :],
                            start=(dt == 0), stop=(dt == 1),
                            tile_position=(0, 32 * il),
                        )
        inter = intp.tile([128, 4, 4, B], F16, tag="inter")
        nc.vector.tensor_copy(inter[:], ip[:])

        # einsum2: acc[k][32 jl + b, d] += inter[:, k, jl, :]^T @ Wt[:, k, jl, :]
        for k in range(4):
            for jl in range(4):
                off = 4096 + (k * 4 + jl) * D
                nc.tensor.matmul(
                    acc[k][32 * jl : 32 * jl + B, :],
                    inter[:, k, jl, :],
                    wcb[:, off : off + D],
                    start=(g == 0), stop=(g == G - 1),
                    tile_position=(0, 32 * jl),
                )

    # ---- relu / residual / LayerNorm, whole quad at once in the
    # accumulator layout [32 jl + b, d] (rows 8-31 of each 32-block are
    # garbage and never read) ----
    hTloc = [ssb.tile([128, JL * B], F16, name=f"hTl{dt}") for dt in range(2)]
    for k in range(4):
        hsum = ssb.tile([128, D], F32, tag="hsum")
        nc.scalar.activation(
            hsum[:], acc[k][:], mybir.ActivationFunctionType.Relu,
        )
        nc.vector.tensor_tensor(
            hsum[:], hsum[:], h_upd[k][:], op=mybir.AluOpType.add
        )
        mean = ssb.tile([128, 1], F32, tag="mean")
        nc.vector.tensor_reduce(
            mean[:], hsum[:], axis=mybir.AxisListType.X, op=mybir.AluOpType.add
        )
        nc.vector.tensor_scalar_mul(mean[:], mean[:], -1.0 / D)
        cen = ssb.tile([128, D], F32, tag="cen")
        nc.vector.tensor_scalar_add(cen[:], hsum[:], mean[:])
        sq = ssb.tile([128, D], F32, tag="sq")
        nc.vector.tensor_tensor(
            sq[:], cen[:], cen[:], op=mybir.AluOpType.mult
        )
        var = ssb.tile([128, 1], F32, tag="var")
        nc.vector.tensor_reduce(
            var[:], sq[:], axis=mybir.AxisListType.X, op=mybir.AluOpType.add
        )
        std = ssb.tile([128, 1], F32, tag="std")
        nc.scalar.activation(
            std[:], var[:], mybir.ActivationFunctionType.Sqrt,
            bias=eps_sb[:], scale=1.0 / D,
        )
        rstd = ssb.tile([128, 1], F32, tag="rstd")
        nc.vector.reciprocal(rstd[:], std[:])
        nc.vector.tensor_scalar_mul(cen[:], cen[:], rstd[:])
        nc.vector.tensor_tensor(
            cen[:], cen[:], lngb_sb[:, t, 0:D], op=mybir.AluOpType.mult
        )
        nc.vector.tensor_tensor(
            h_upd[k][:], cen[:], lngb_sb[:, t, D : 2 * D],
            op=mybir.AluOpType.add,
        )
        # transpose the quad's updated slots into hTloc (f16 store):
        # one full 128x128 transpose per d-half; cols (32 jloc + b) then
        # pack into hTloc's (jl, b) order.
        for dt in range(2):
            p3 = tpp.tile([128, 128], F32, tag="p3")
            nc.tensor.transpose(
                p3[:], h_upd[k][:, 128 * dt : 128 * (dt + 1)], ident[:]
            )
            for jloc in range(4):
                jl = 4 * k + jloc
                nc.vector.tensor_copy(
                    hTloc[dt][:, jl * B : (jl + 1) * B],
                    p3[:, 32 * jloc : 32 * jloc + B],
                )

    # ---- AllGather the transposed updated slots (fp16); rebuild hT ----
    agin = drp.tile([2 * 128 * JL * B], F16, tag=f"agin{t}")
    for dt in range(2):
        nc.sync.dma_start(
            agin[dt * 16384 : (dt + 1) * 16384].rearrange(
                "(p f) -> p f", p=128
            ),
            hTloc[dt][:],
        )
    agout = drp.tile([NC, 2 * 128 * JL * B], F16, addr_space="Shared",
                     tag=f"agout{t}")
    nc.gpsimd.collective_compute(
        "AllGather", mybir.AluOpType.bypass,
        ins=[agin[:].opt()], outs=[agout[:].opt()],
        replica_groups=[list(range(NC))],
    )
    agr = agout[:].rearrange(
        "rk (dt dp jl b) -> dt dp rk jl b", dt=2, dp=128, jl=JL, b=B
    )
    for dt in range(2):
        nc.sync.dma_start(
            hT[dt][:].rearrange("dp (rk jl) b -> dp rk jl b", rk=NC), agr[dt]
        )


def _phase_c(nc, tc, io, ident, pid, hT, qoT):
    with tc.tile_pool(name="pc_sb", bufs=1) as pc, \
         tc.tile_pool(name="pc_ps", bufs=3, space="PSUM") as cps, \
         tc.tile_pool(name="pc_lg", bufs=4, space="PSUM") as lgps, \
         tc.tile_pool(name="pc_wo", bufs=4) as wop:

        wkf_sb = [pc.tile([128, D], F32, name=f"wkf{ct}") for ct in range(2)]
        wvf_sb = [pc.tile([128, D], F32, name=f"wvf{ct}") for ct in range(2)]
        for ct in range(2):
            nc.sync.dma_start(
                wkf_sb[ct][:], io["wkfT"][128 * ct : 128 * (ct + 1), :]
            )
            nc.sync.dma_start(
                wvf_sb[ct][:], io["wvfT"][128 * ct : 128 * (ct + 1), :]
            )

        # own-batch h^T slice (dynamic b=pid) -> static tiles
        pid_v = nc.vector.partition_id()
        hb = [pc.tile([128, S], F32, name=f"hb{dt}") for dt in range(2)]
        for dt in range(2):
            nc.vector.tensor_copy(
                hb[dt][:].rearrange("p (s o) -> p s o", o=1),
                hT[dt][:, :, bass.ds(pid_v, 1)],
            )

        # K_f^T [d'128 x2, s128] ; V_f natural [s, d']
        kfT = [pc.tile([128, S], F32, name=f"kfT{pt}") for pt in range(2)]
        for pt in range(2):
            ps = cps.tile([128, L], F32, tag="c")
            for ct in range(2):
                nc.tensor.matmul(
                    ps[:, 0:S], wkf_sb[ct][:, 128 * pt : 128 * (pt + 1)], hb[ct][:],
                    start=(ct == 0), stop=(ct == 1),
                )
            nc.vector.tensor_copy(kfT[pt][:], ps[:, 0:S])
        vf = pc.tile([S, D], F32)
        psv = cps.tile([128, L], F32, tag="c")
        for ct in range(2):
            nc.tensor.matmul(
                psv[0:S, 0:D], hb[ct][:], wvf_sb[ct][:],
                start=(ct == 0), stop=(ct == 1),
            )
        nc.vector.tensor_copy(vf[:], psv[0:S, 0:D])

        # expand attention -> A2^T [s, t512]
        a2T = pc.tile([S, L], F32)
        for tt in range(4):
            sc = cps.tile([128, L], F32, tag="c")
            for pt in range(2):
                nc.tensor.matmul(
                    sc[:, 0:S], qoT[pt][:, 128 * tt : 128 * (tt + 1)], kfT[pt][:],
                    start=(pt == 0), stop=(pt == 1),
                )
            rowmax = pc.tile([128, 1], F32, tag="rmax2")
            nc.vector.tensor_reduce(
                rowmax[:], sc[:, 0:S], axis=mybir.AxisListType.X,
                op=mybir.AluOpType.max,
            )
            nb = pc.tile([128, 1], F32, tag="nb2")
            nc.vector.tensor_scalar_mul(nb[:], rowmax[:], -SCALE)
            a2 = pc.tile([128, S], F32, tag="a2")
            sumexp = pc.tile([128, 1], F32, tag="sexp2")
            nc.scalar.activation(
                a2[:], sc[:, 0:S], mybir.ActivationFunctionType.Exp,
                bias=nb[:], scale=SCALE, accum_out=sumexp[:],
            )
            rs = pc.tile([128, 1], F32, tag="rs2")
            nc.vector.reciprocal(rs[:], sumexp[:])
            nc.vector.tensor_scalar_mul(a2[:], a2[:], rs[:])
            ptr = cps.tile([128, L], F32, tag="c")
            nc.tensor.transpose(ptr[:, 0:S], a2[:], ident[:])
            nc.vector.tensor_copy(a2T[:, 128 * tt : 128 * (tt + 1)], ptr[:, 0:S])

        # Y^T [d128 x2, t512] -> fp16 for the vocab matmuls
        yTh = [pc.tile([128, L], F16, name=f"yTh{dt}") for dt in range(2)]
        for dt in range(2):
            ps = cps.tile([128, L], F32, tag="c")
            nc.tensor.matmul(
                ps[:], vf[:, 128 * dt : 128 * (dt + 1)], a2T[:],
                start=True, stop=True,
            )
            nc.vector.tensor_copy(yTh[dt][:], ps[:])

        # logits tiles: 1 MB W_out blocks (4 vocab tiles), fp16 batched stores
        lg_outr = io["lg_out"].rearrange("(tt p) v -> p tt v", tt=4)
        for bv in range(NVB):
            wo_sb = wop.tile([128, 2, 4, VT], F16, tag="wo")
            nc.scalar.dma_start(
                wo_sb[:],
                io["woutT"][bv].rearrange("dt dp (q v) -> dp dt q v", q=4),
            )
            for q in range(4):
                vt = 4 * bv + q
                lg_sb = wop.tile([128, 4, VT], F16, tag="lg_sb", name="lg_sb")
                for tt in range(4):
                    lg = lgps.tile([128, VT], F32, tag="lg")
                    for dt in range(2):
                        nc.tensor.matmul(
                            lg[:],
                            yTh[dt][:, 128 * tt : 128 * (tt + 1)],
                            wo_sb[:, dt, q, :],
                            start=(dt == 0), stop=(dt == 1),
                        )
                    nc.any.tensor_copy(lg_sb[:, tt, :], lg[:])
                nc.sync.dma_start(
                    lg_outr[:, :, VT * vt : VT * (vt + 1)], lg_sb[:]
                )


# ---------------------------------------------------------------------------
# Host side
# ---------------------------------------------------------------------------

_NC_CACHE = {}


def _get_nc():
    key = N_STEPS
    if key not in _NC_CACHE:
        _NC_CACHE[key] = build()
    return _NC_CACHE[key]


def _prep_in_maps(inputs):
    f32 = lambda a: np.ascontiguousarray(np.asarray(a), dtype=np.float32)
    input_ids = np.asarray(inputs["input_ids"])
    attention_mask = np.asarray(inputs["attention_mask"])
    H = f32(inputs["H"])
    W_source = f32(inputs["W_source"])
    W_target = f32(inputs["W_target"])

    lngb = np.zeros((128, STEPS, 2 * D), dtype=np.float32)
    lngb[:, :, 0:D] = np.asarray(inputs["ln_scale"])[None]
    lngb[:, :, D:] = np.asarray(inputs["ln_bias"])[None]

    rep = {
        "wqT": f32(np.asarray(inputs["Wq_in"]).T),
        "wkslT": f32(np.asarray(inputs["Wk_slots"]).T),
        "wvT": f32(np.asarray(inputs["Wv_in"]).T),
        "wqoT": f32(np.asarray(inputs["Wq_out"]).T),
        "wkfT": f32(np.asarray(inputs["Wk_fin"]).T),
        "wvfT": f32(np.asarray(inputs["Wv_fin"]).T),
        "hT_in": f32(H.T),
        "hn_in": H,
        "lngb": lngb,
        # woutT[bv, dtile, dp, (q vl)] = Wout[500(4bv+q)+vl, 128dt+dp]
        "woutT": np.ascontiguousarray(
            f32(inputs["W_out_proj"])
            .reshape(NVB, 4, VT, 2, 128)
            .transpose(0, 3, 4, 1, 2)
            .reshape(NVB, 2, 128, 4 * VT)
        ).astype(np.float16),
    }

    in_maps = []
    for c in range(NC):
        m = dict(rep)
        X = (np.asarray(inputs["token_emb"], dtype=np.float32)[input_ids[c]]
             + np.asarray(inputs["pos_emb"], dtype=np.float32))
        m["xT_in"] = np.ascontiguousarray(X.T)
        m["maskw"] = np.ascontiguousarray(
            attention_mask[c].astype(np.float32).reshape(4, 128).T
        )
        # wsrc[g, dp, (dt il k jloc r)] = W_source[4g+il, 16c+4k+jloc, 128dt+dp, r]
        ws = W_source[:, JL * c : JL * (c + 1)]      # [S, 16, D, R]
        ws = ws.reshape(G, 4, 4, 4, 2, 128, R).transpose(0, 5, 4, 1, 2, 3, 6)
        ws = np.ascontiguousarray(ws).astype(np.float16).reshape(G, 128, 4096)
        # wtgt[g, (il r), (k jloc d)] = W_target[4g+il, 16c+4k+jloc, r, d]
        # The reference masks out the i == j (diagonal) pair; zeroing
        # W_target[j, j] is exactly equivalent since the term is linear in it.
        wt = W_target[:, JL * c : JL * (c + 1)].copy()   # [S, 16, R, D]
        for jl in range(JL):
            wt[JL * c + jl, jl] = 0.0
        wt = wt.reshape(G, 4, 4, 4, R, D).transpose(0, 1, 4, 2, 3, 5)
        wt = np.ascontiguousarray(wt).astype(np.float16).reshape(G, 128, 4096)
        m["wcomb"] = np.ascontiguousarray(
            np.concatenate([ws, wt], axis=2)
        )
        in_maps.append(m)
    return in_maps


def run(inputs, trace=False):
    nc = _get_nc()
    in_maps = _prep_in_maps(inputs)
    res = bass_utils.run_bass_kernel_spmd(
        nc, in_maps, core_ids=list(range(NC)), trace=trace
    )
    out = np.stack(
        [res.results[c]["lg_out"].astype(np.float32) for c in range(NC)], axis=0
    )
    return out, res


def kernel(**inputs):
    out, _ = run(inputs, trace=False)
    return out
